# revision 4
# baseline (speedup 1.0000x reference)
"""Trainium2 Bass kernel v7 for nn_ModalDecoder (embedding_lookup).

Per core (8 = 4 b x 2 N-halves):
    y[fo, n] = Wpack[d, fo]^T @ xT[d, n] + cvec[fo]
with bf16 wire dtypes, fp32 PSUM accumulation, bf16 output.

v3 insights (from HW traces of baseline + v2 + probes):
  - The HAM activity controller grants 100% utilization (PE 2.4GHz + full
    DMA rate) only after ~3.7us of sustained engine activity; until then
    everything runs at ~50%. Warmup matmuls are issued at MAIN scope
    (before Block entry) so the grant lands at ~10.8us, right when the
    first gating load completes.
  - Loads on ONE sync HWDGE ring (dual-queue aggregate measured slower).
    128KB gating chunks (sem receipt ~0.65us vs ~1.15us for 256KB). Ring
    order gives the PE a 512KB prefix (xt01, wp0, wp1) so k01 work for
    groups 0-1 covers the arrival gap of the remaining xt chunks.
  - Output bf16, y layout [128, ST*NH]; three progressive stores (512KB /
    384KB / 128KB); the last one's data flight hides under the NEFF's
    fixed ~7us semaphore-sweep epilogue.
  - Last group's bias-add split in half across DVE and ACT.
"""

import numpy as np
import ml_dtypes

B, N, D, O, F, V = 4, 1024, 512, 64, 16, 64
NH = N // 2
FO = F * O
KT = D // 128
ST = FO // 128
N_WARM = 9

_cache: dict = {}


def _build_program(with_clears=True):
    import concourse.bass as bass
    import concourse.mybir as mybir
    from contextlib import ExitStack

    bf16 = mybir.dt.bfloat16
    f32 = mybir.dt.float32
    ACT_IDENT = mybir.ActivationFunctionType.Identity

    nc = bass.Bass(
        "TRN2",
        target_bir_lowering=False,
        debug=False,
        num_devices=8,
        detect_race_conditions=not with_clears,
    )

    xt_d = nc.dram_tensor("xt", [128, KT * NH], bf16, kind="ExternalInput")
    wp_d = nc.dram_tensor("wp", [128, KT * FO], bf16, kind="ExternalInput")
    cv_d = nc.dram_tensor("cv", [128, ST], f32, kind="ExternalInput")
    y_d = nc.dram_tensor("y", [128, ST * NH], bf16, kind="ExternalOutput")

    es = ExitStack()
    xt_sb = es.enter_context(nc.sbuf_tensor("xt_sb", [128, KT * NH], bf16))
    wp_sb = es.enter_context(nc.sbuf_tensor("wp_sb", [128, KT * FO], bf16))
    cv_sb = es.enter_context(nc.sbuf_tensor("cv_sb", [128, ST], f32))
    out_sb = es.enter_context(nc.sbuf_tensor("out_sb", [128, ST * NH], bf16))
    scr_sb = es.enter_context(nc.sbuf_tensor("scr_sb", [128, NH], bf16))
    ps = es.enter_context(nc.psum_tensor([128, ST, NH], f32))
    s_xt01 = es.enter_context(nc.semaphore("s_xt01"))
    s_xt23 = es.enter_context(nc.semaphore("s_xt23"))
    s_wp0 = es.enter_context(nc.semaphore("s_wp0"))
    s_wp1 = es.enter_context(nc.semaphore("s_wp1"))
    s_wp2 = es.enter_context(nc.semaphore("s_wp2"))
    s_wp4 = es.enter_context(nc.semaphore("s_wp4"))
    s_wp6 = es.enter_context(nc.semaphore("s_wp6"))
    s_cv = es.enter_context(nc.semaphore("s_cv"))
    s_ws = es.enter_context(nc.semaphore("s_ws"))
    s_mm = es.enter_context(nc.semaphore("s_mm"))
    s_add = es.enter_context(nc.semaphore("s_add"))
    s_st = es.enter_context(nc.semaphore("s_st"))

    # ---- main-scope load triggers: the baseline's proven 7-DMA ring ----
    # xt01 (256KB), wp0 (128KB), xt23 (256KB), wp1 (128KB), wp23, wp45,
    # wp67 (256KB each, 2KB descriptors).
    nc.sync.dma_start(xt_sb[:, 0:1024], xt_d.ap()[:, 0:1024]).then_inc(s_xt01, 16)
    nc.sync.dma_start(wp_sb[:, 0:512], wp_d.ap()[:, 0:512]).then_inc(s_wp0, 16)
    nc.sync.dma_start(xt_sb[:, 1024:2048], xt_d.ap()[:, 1024:2048]).then_inc(s_xt23, 16)
    nc.sync.dma_start(wp_sb[:, 1024:2048], wp_d.ap()[:, 1024:2048]).then_inc(s_wp2, 16)
    nc.sync.dma_start(wp_sb[:, 512:1024], wp_d.ap()[:, 512:1024]).then_inc(s_wp1, 16)
    nc.sync.dma_start(wp_sb[:, 2048:3072], wp_d.ap()[:, 2048:3072]).then_inc(s_wp4, 16)
    nc.sync.dma_start(wp_sb[:, 3072:4096], wp_d.ap()[:, 3072:4096]).then_inc(s_wp6, 16)
    # cv on the otherwise-idle scalar ring
    nc.scalar.dma_start(cv_sb[:], cv_d.ap()).then_inc(s_cv, 16)

    # ---- main-scope PE warmup (HW only): earns the HAM 100%-util grant
    # (~3.7us of sustained activity) before the first gating load lands.
    # The sim variant keeps warmups in-block, gated on the DVE memset
    # (CoreSim rejects reads of uninitialized SBUF).
    if with_clears:
        # DVE activity in parallel with PE warmups pulls the HAM grant
        # earlier (the activity monitor aggregates engines)
        for _ in range(8):
            nc.vector.memset(scr_sb[:], 0)
        for _ in range(N_WARM):
            nc.tensor.matmul(
                ps[:, ST - 1, :], scr_sb[:, :128], scr_sb[:], start=True, stop=True
            )

    with nc.Block() as block:

        @block.sync
        def _(sync):
            sync.wait_ge(s_add, 4)
            sync.dma_start(y_d.ap()[:, 0:2048], out_sb[:, 0:2048]).then_inc(s_st, 16)
            sync.wait_ge(s_add, 7)
            sync.dma_start(y_d.ap()[:, 2048:3584], out_sb[:, 2048:3584]).then_inc(
                s_st, 16
            )
            sync.wait_ge(s_add, 9)
            sync.dma_start(y_d.ap()[:, 3584:4096], out_sb[:, 3584:4096]).then_inc(
                s_st, 16
            )

        @block.tensor
        def _(tensor):
            if not with_clears:
                tensor.wait_ge(s_ws, 1)
                for _ in range(N_WARM):
                    nc.tensor.matmul(
                        ps[:, ST - 1, :], scr_sb[:, :128], scr_sb[:],
                        start=True, stop=True,
                    )

            def mm(s, k):
                return nc.tensor.matmul(
                    ps[:, s, :],
                    wp_sb[:, s * 512 + k * 128:s * 512 + (k + 1) * 128],
                    xt_sb[:, k * NH:(k + 1) * NH],
                    start=(k == 0),
                    stop=(k == KT - 1),
                )

            # g0 k01 on the prefix; k23 when xt23 lands; then groups in
            # ring-arrival order (wp23 precedes wp1 in the ring)
            tensor.wait_ge(s_xt01, 16)
            tensor.wait_ge(s_wp0, 16)
            mm(0, 0); mm(0, 1)
            tensor.wait_ge(s_xt23, 16)
            mm(0, 2)
            mm(0, 3).then_inc(s_mm, 1)
            for s, sem in ((2, s_wp2), (3, None), (1, s_wp1),
                           (4, s_wp4), (5, None), (6, s_wp6)):
                if sem is not None:
                    tensor.wait_ge(sem, 16)
                mm(s, 0); mm(s, 1); mm(s, 2)
                mm(s, 3).then_inc(s_mm, 1)
            # group 7 in n-halves: h0's bias-add overlaps h1's matmuls,
            # leaving only a 256-col add on the critical tail
            for h in range(2):
                for k in range(KT):
                    inst = nc.tensor.matmul(
                        ps[:, 7, h * 256:(h + 1) * 256],
                        wp_sb[:, 7 * 512 + k * 128:7 * 512 + (k + 1) * 128],
                        xt_sb[:, k * NH + h * 256:k * NH + (h + 1) * 256],
                        start=(k == 0),
                        stop=(k == KT - 1),
                    )
                    if k == KT - 1:
                        inst.then_inc(s_mm, 1)

        @block.vector
        def _(vector):
            if not with_clears:
                vector.memset(scr_sb[:], 0).then_inc(s_ws, 1)
            vector.wait_ge(s_cv, 16)
            for j, s in enumerate((0, 2, 3, 1, 4, 5, 6)):
                vector.wait_ge(s_mm, j + 1)
                nc.vector.tensor_scalar_add(
                    out_sb[:, s * NH:(s + 1) * NH], ps[:, s, :],
                    cv_sb[:, s:s + 1],
                ).then_inc(s_add, 1)
            s = ST - 1
            for h in range(2):
                vector.wait_ge(s_mm, ST + h)
                nc.vector.tensor_scalar_add(
                    out_sb[:, s * NH + h * 256:s * NH + (h + 1) * 256],
                    ps[:, s, h * 256:(h + 1) * 256],
                    cv_sb[:, s:s + 1],
                ).then_inc(s_add, 1)

    es.close()

    return nc


def _get_program():
    nc = _cache.get("nc")
    if nc is None:
        nc = _build_program()
        _cache["nc"] = nc
    return nc


def _prep_in_maps(x, idx, fbt, opt):
    bf = ml_dtypes.bfloat16
    in_maps = []
    for b in range(B):
        w = opt[idx[b]].reshape(F, D, O)                     # [F,D,O] f32
        wpack = w.transpose(1, 0, 2).reshape(KT, 128, ST, 128)  # [k,p,s,c]
        wp_host = np.ascontiguousarray(
            wpack.transpose(1, 2, 0, 3).reshape(128, KT * FO)
        ).astype(bf)                                         # [p, s*512+k*128+c]
        bias = fbt[idx[b]]                                   # [F,D]
        cvec = np.einsum("fd,fdo->fo", bias, w).reshape(FO).astype(np.float32)
        cv = np.ascontiguousarray(cvec.reshape(ST, 128).T)   # [128, ST]
        for h in range(2):
            xtT = x[b, h * NH:(h + 1) * NH, :].T             # [D, NH]
            xt_host = np.ascontiguousarray(
                xtT.reshape(KT, 128, NH).transpose(1, 0, 2).reshape(128, KT * NH)
            ).astype(bf)                                     # [128, KT*NH]
            in_maps.append({"xt": xt_host, "wp": wp_host, "cv": cv})
    return in_maps


def _assemble(results):
    out = np.empty((B, N, F, O), dtype=np.float32)
    for c in range(8):
        b, h = divmod(c, 2)
        y = np.asarray(results[c]["y"]).astype(np.float32)   # [128, ST*NH]
        fo_n = y.reshape(128, ST, NH).transpose(1, 0, 2).reshape(FO, NH)
        out[b, h * NH:(h + 1) * NH] = fo_n.reshape(F, O, NH).transpose(2, 0, 1)
    return out


def _run(x, idx, feature_bias_table, out_projection_table, **run_kwargs):
    from concourse.bass_utils import run_bass_kernel_spmd

    x = np.asarray(x, dtype=np.float32)
    idx = np.asarray(idx).astype(np.int64)
    fbt = np.asarray(feature_bias_table, dtype=np.float32)
    opt = np.asarray(out_projection_table, dtype=np.float32)

    nc = _get_program()
    in_maps = _prep_in_maps(x, idx, fbt, opt)
    res = run_bass_kernel_spmd(nc, in_maps, core_ids=list(range(8)), **run_kwargs)
    return _assemble(res.results), res


def kernel(x, idx, feature_bias_table, out_projection_table):
    out, _ = _run(x, idx, feature_bias_table, out_projection_table)
    return out


# revision 5
# speedup vs baseline: 1.1502x; 1.1502x over previous
"""Trainium2 Bass kernel v7 for nn_ModalDecoder (embedding_lookup).

Per core (8 = 4 b x 2 N-halves):
    y[fo, n] = Wpack[d, fo]^T @ xT[d, n] + cvec[fo]
with bf16 wire dtypes, fp32 PSUM accumulation, bf16 output.

v3 insights (from HW traces of baseline + v2 + probes):
  - The HAM activity controller grants 100% utilization (PE 2.4GHz + full
    DMA rate) only after ~3.7us of sustained engine activity; until then
    everything runs at ~50%. Warmup matmuls are issued at MAIN scope
    (before Block entry) so the grant lands at ~10.8us, right when the
    first gating load completes.
  - Loads on ONE sync HWDGE ring (dual-queue aggregate measured slower).
    128KB gating chunks (sem receipt ~0.65us vs ~1.15us for 256KB). Ring
    order gives the PE a 512KB prefix (xt01, wp0, wp1) so k01 work for
    groups 0-1 covers the arrival gap of the remaining xt chunks.
  - Output bf16, y layout [128, ST*NH]; three progressive stores (512KB /
    384KB / 128KB); the last one's data flight hides under the NEFF's
    fixed ~7us semaphore-sweep epilogue.
  - Last group's bias-add split in half across DVE and ACT.
"""

import numpy as np
import ml_dtypes

B, N, D, O, F, V = 4, 1024, 512, 64, 16, 64
NH = N // 2
FO = F * O
KT = D // 128
ST = FO // 128
N_WARM = 9

_cache: dict = {}


def _build_program(with_clears=True):
    import concourse.bass as bass
    import concourse.mybir as mybir
    from contextlib import ExitStack

    bf16 = mybir.dt.bfloat16
    f32 = mybir.dt.float32
    ACT_IDENT = mybir.ActivationFunctionType.Identity

    nc = bass.Bass(
        "TRN2",
        target_bir_lowering=False,
        debug=False,
        num_devices=8,
        detect_race_conditions=not with_clears,
    )

    xt_d = nc.dram_tensor("xt", [128, KT * NH], bf16, kind="ExternalInput")
    wp_d = nc.dram_tensor("wp", [128, KT * FO], bf16, kind="ExternalInput")
    cv_d = nc.dram_tensor("cv", [128, ST], f32, kind="ExternalInput")
    y_d = nc.dram_tensor("y", [128, ST * NH], bf16, kind="ExternalOutput")

    es = ExitStack()
    xt_sb = es.enter_context(nc.sbuf_tensor("xt_sb", [128, KT * NH], bf16))
    wp_sb = es.enter_context(nc.sbuf_tensor("wp_sb", [128, KT * FO], bf16))
    cv_sb = es.enter_context(nc.sbuf_tensor("cv_sb", [128, ST], f32))
    out_sb = es.enter_context(nc.sbuf_tensor("out_sb", [128, ST * NH], bf16))
    scr_sb = es.enter_context(nc.sbuf_tensor("scr_sb", [128, NH], bf16))
    ps = es.enter_context(nc.psum_tensor([128, ST, NH], f32))
    s_xt01 = es.enter_context(nc.semaphore("s_xt01"))
    s_xt23 = es.enter_context(nc.semaphore("s_xt23"))
    s_wp0 = es.enter_context(nc.semaphore("s_wp0"))
    s_wp1 = es.enter_context(nc.semaphore("s_wp1"))
    s_wp2 = es.enter_context(nc.semaphore("s_wp2"))
    s_wp4 = es.enter_context(nc.semaphore("s_wp4"))
    s_wp6 = es.enter_context(nc.semaphore("s_wp6"))
    s_cv = es.enter_context(nc.semaphore("s_cv"))
    s_ws = es.enter_context(nc.semaphore("s_ws"))
    s_mm = es.enter_context(nc.semaphore("s_mm"))
    s_add = es.enter_context(nc.semaphore("s_add"))
    s_st = es.enter_context(nc.semaphore("s_st"))

    # ---- main-scope load triggers: the baseline's proven 7-DMA ring ----
    # xt01 (256KB), wp0 (128KB), xt23 (256KB), wp1 (128KB), wp23, wp45,
    # wp67 (256KB each, 2KB descriptors).
    nc.sync.dma_start(xt_sb[:, 0:1024], xt_d.ap()[:, 0:1024]).then_inc(s_xt01, 16)
    nc.sync.dma_start(wp_sb[:, 0:512], wp_d.ap()[:, 0:512]).then_inc(s_wp0, 16)
    nc.sync.dma_start(xt_sb[:, 1024:2048], xt_d.ap()[:, 1024:2048]).then_inc(s_xt23, 16)
    nc.sync.dma_start(wp_sb[:, 1024:2048], wp_d.ap()[:, 1024:2048]).then_inc(s_wp2, 16)
    nc.sync.dma_start(wp_sb[:, 512:1024], wp_d.ap()[:, 512:1024]).then_inc(s_wp1, 16)
    nc.sync.dma_start(wp_sb[:, 2048:3072], wp_d.ap()[:, 2048:3072]).then_inc(s_wp4, 16)
    nc.sync.dma_start(wp_sb[:, 3072:4096], wp_d.ap()[:, 3072:4096]).then_inc(s_wp6, 16)
    # cv on the otherwise-idle scalar ring
    nc.scalar.dma_start(cv_sb[:], cv_d.ap()).then_inc(s_cv, 16)

    # ---- main-scope PE warmup (HW only): earns the HAM 100%-util grant
    # (~3.7us of sustained activity) before the first gating load lands.
    # The sim variant keeps warmups in-block, gated on the DVE memset
    # (CoreSim rejects reads of uninitialized SBUF).
    if with_clears:
        # DVE activity in parallel with PE warmups pulls the HAM grant
        # earlier (the activity monitor aggregates engines)
        for _ in range(8):
            nc.vector.memset(scr_sb[:], 0)
        for _ in range(N_WARM):
            nc.tensor.matmul(
                ps[:, ST - 1, :], scr_sb[:, :128], scr_sb[:], start=True, stop=True
            )

    with nc.Block() as block:

        @block.sync
        def _(sync):
            sync.wait_ge(s_add, 4)
            sync.dma_start(y_d.ap()[:, 0:2048], out_sb[:, 0:2048]).then_inc(s_st, 16)
            sync.wait_ge(s_add, 7)
            sync.dma_start(y_d.ap()[:, 2048:3584], out_sb[:, 2048:3584]).then_inc(
                s_st, 16
            )
            sync.wait_ge(s_add, 8)
            sync.dma_start(y_d.ap()[:, 3584:4096], out_sb[:, 3584:4096]).then_inc(
                s_st, 16
            )

        @block.tensor
        def _(tensor):
            if not with_clears:
                tensor.wait_ge(s_ws, 1)
                for _ in range(N_WARM):
                    nc.tensor.matmul(
                        ps[:, ST - 1, :], scr_sb[:, :128], scr_sb[:],
                        start=True, stop=True,
                    )

            def mm(s, k):
                return nc.tensor.matmul(
                    ps[:, s, :],
                    wp_sb[:, s * 512 + k * 128:s * 512 + (k + 1) * 128],
                    xt_sb[:, k * NH:(k + 1) * NH],
                    start=(k == 0),
                    stop=(k == KT - 1),
                )

            # g0 k01 on the prefix; k23 when xt23 lands; then groups in
            # ring-arrival order (wp23 precedes wp1 in the ring)
            tensor.wait_ge(s_xt01, 16)
            tensor.wait_ge(s_wp0, 16)
            mm(0, 0); mm(0, 1)
            tensor.wait_ge(s_xt23, 16)
            mm(0, 2)
            mm(0, 3).then_inc(s_mm, 1)
            for s, sem in ((2, s_wp2), (3, None), (1, s_wp1),
                           (4, s_wp4), (5, None), (6, s_wp6), (7, None)):
                if sem is not None:
                    tensor.wait_ge(sem, 16)
                mm(s, 0); mm(s, 1); mm(s, 2)
                mm(s, 3).then_inc(s_mm, 1)

        @block.vector
        def _(vector):
            if not with_clears:
                vector.memset(scr_sb[:], 0).then_inc(s_ws, 1)
            vector.wait_ge(s_cv, 16)
            for j, s in enumerate((0, 2, 3, 1, 4, 5, 6)):
                vector.wait_ge(s_mm, j + 1)
                nc.vector.tensor_scalar_add(
                    out_sb[:, s * NH:(s + 1) * NH], ps[:, s, :],
                    cv_sb[:, s:s + 1],
                ).then_inc(s_add, 1)
            s = ST - 1
            vector.wait_ge(s_mm, ST)
            nc.vector.tensor_scalar_add(
                out_sb[:, s * NH:(s + 1) * NH], ps[:, s, :],
                cv_sb[:, s:s + 1],
            ).then_inc(s_add, 1)

    es.close()

    return nc


def _get_program():
    nc = _cache.get("nc")
    if nc is None:
        nc = _build_program()
        _cache["nc"] = nc
    return nc


def _prep_in_maps(x, idx, fbt, opt):
    bf = ml_dtypes.bfloat16
    in_maps = []
    for b in range(B):
        w = opt[idx[b]].reshape(F, D, O)                     # [F,D,O] f32
        wpack = w.transpose(1, 0, 2).reshape(KT, 128, ST, 128)  # [k,p,s,c]
        wp_host = np.ascontiguousarray(
            wpack.transpose(1, 2, 0, 3).reshape(128, KT * FO)
        ).astype(bf)                                         # [p, s*512+k*128+c]
        bias = fbt[idx[b]]                                   # [F,D]
        cvec = np.einsum("fd,fdo->fo", bias, w).reshape(FO).astype(np.float32)
        cv = np.ascontiguousarray(cvec.reshape(ST, 128).T)   # [128, ST]
        for h in range(2):
            xtT = x[b, h * NH:(h + 1) * NH, :].T             # [D, NH]
            xt_host = np.ascontiguousarray(
                xtT.reshape(KT, 128, NH).transpose(1, 0, 2).reshape(128, KT * NH)
            ).astype(bf)                                     # [128, KT*NH]
            in_maps.append({"xt": xt_host, "wp": wp_host, "cv": cv})
    return in_maps


def _assemble(results):
    out = np.empty((B, N, F, O), dtype=np.float32)
    for c in range(8):
        b, h = divmod(c, 2)
        y = np.asarray(results[c]["y"]).astype(np.float32)   # [128, ST*NH]
        fo_n = y.reshape(128, ST, NH).transpose(1, 0, 2).reshape(FO, NH)
        out[b, h * NH:(h + 1) * NH] = fo_n.reshape(F, O, NH).transpose(2, 0, 1)
    return out


def _run(x, idx, feature_bias_table, out_projection_table, **run_kwargs):
    from concourse.bass_utils import run_bass_kernel_spmd

    x = np.asarray(x, dtype=np.float32)
    idx = np.asarray(idx).astype(np.int64)
    fbt = np.asarray(feature_bias_table, dtype=np.float32)
    opt = np.asarray(out_projection_table, dtype=np.float32)

    nc = _get_program()
    in_maps = _prep_in_maps(x, idx, fbt, opt)
    res = run_bass_kernel_spmd(nc, in_maps, core_ids=list(range(8)), **run_kwargs)
    return _assemble(res.results), res


def kernel(x, idx, feature_bias_table, out_projection_table):
    out, _ = _run(x, idx, feature_bias_table, out_projection_table)
    return out
